# revision 22
# baseline (speedup 1.0000x reference)
"""Plane-sweep cost-volume kernel for Trainium2 (8 NeuronCores).

Problem shape (hardcoded): B=1, V=4 source views, C=16 feature channels,
H=64, W=96, D=64 depth planes.  Output: (1, D, H, W) float32.

Strategy
--------
The benchmark geometry has identity rotations (extrinsics are pure
translations) and zero-skew pinhole intrinsics, so for each (view, depth
plane) the warp from output pixels to source-image sample coordinates is an
axis-separable affine map:  x = ax + bx*px,  y = ay + by*py.  Bilinear
grid_sample with zero padding then factorizes exactly into two 1-D linear
interpolations, each a small dense matrix of "hat" functions
hat(t - k) = max(0, 1 - |t - k|):

    warped_c = Ay(v,d) @ src_c @ Bx(v,d)^T

so the whole cost volume becomes TensorEngine matmuls -- no gathers.  The
hat matrices are built on the host (bf16), as are the repacked bf16 feature
layouts, so the device only runs matmuls plus the PSUM->SBUF drains and the
final per-pixel channel dot.

Engine schedule (per core, 8 depth planes):
  stage 1 (y-interp): 64 matmuls N=512; PSUM drains alternate between the
    Scalar and Vector engines.
  stage 2 (x-interp + view-sum in PSUM): 64 matmuls N=512 streamed with
    channels innermost, so the channel dot is: ACT drain (bf16), DVE
    multiply (bf16 2x mode), then an innermost-8 reduce via DVE pool_avg
    (even planes) or a GPSIMD add-tree (odd planes).
  A burst of dummy warm-up matmuls runs during the input DMA window to lift
  the PE clock gate (HAM) to 2.4 GHz before real work starts.

Sharding: depth planes across the 8 cores (8 planes each); features are
replicated.  If the inputs do not have the separable structure we fall back
to an exact numpy implementation.
"""

import numpy as np

H, W, D, V, C = 64, 96, 64, 4, 16
N_CORES = 8
DLOC = D // N_CORES            # 8 depth planes per core
EPS = 1e-8
OOB = 1.0e9                    # sample coord pushed out of range => zero weights

_CACHE = {}

# Stage-1 drain split: True -> scalar engine (ACT), False -> vector (DVE).
_S1_ACT = [i % 2 == 0 for i in range(V * C // 2)]
_N_WARMUP = 14


# --------------------------------------------------------------------------
# Device kernel
# --------------------------------------------------------------------------
def _build_nc():
    import concourse.bacc as bacc
    import concourse.tile as tile
    from concourse import mybir

    fp32 = mybir.dt.float32
    bf16 = mybir.dt.bfloat16
    Alu = mybir.AluOpType

    nc = bacc.Bacc("TRN2", target_bir_lowering=False, debug=False,
                   num_devices=N_CORES)

    srcb = nc.dram_tensor("srcb", [H, V * C * W], bf16, kind="ExternalInput")
    ay = nc.dram_tensor("ay", [H, V * DLOC * H], bf16, kind="ExternalInput")
    bx = nc.dram_tensor("bx", [W, V * DLOC * W], bf16, kind="ExternalInput")
    curb = nc.dram_tensor("curb", [W, C, H], bf16, kind="ExternalInput")
    out = nc.dram_tensor("out", [W, DLOC, H], fp32, kind="ExternalOutput")

    with tile.TileContext(nc) as tc:
        with (
            tc.tile_pool(name="consts", bufs=1) as consts,
            tc.tile_pool(name="psA", bufs=2, space="PSUM") as psA,
            tc.tile_pool(name="psB", bufs=2, space="PSUM") as psB,
            tc.tile_pool(name="q0p", bufs=2) as q0p,
            tc.tile_pool(name="q1p", bufs=2) as q1p,
            tc.tile_pool(name="rp", bufs=2) as rp,
            tc.tile_pool(name="rp2", bufs=2) as rp2,
            tc.tile_pool(name="op", bufs=2) as op,
        ):
            # ---- input DMAs ---------------------------------------------
            sb_ay = consts.tile([H, V * DLOC * H], bf16, tag="ay")
            nc.sync.dma_start(out=sb_ay, in_=ay.ap())
            sb_src = consts.tile([H, V * C * W], bf16, tag="src")
            sl0 = slice(0, C * W)
            nc.sync.dma_start(out=sb_src[:, sl0], in_=srcb.ap()[:, sl0])
            sl1 = slice(C * W, V * C * W)
            nc.sync.dma_start(out=sb_src[:, sl1], in_=srcb.ap()[:, sl1])
            sb_bx = consts.tile([W, V * DLOC * W], bf16, tag="bx")
            nc.sync.dma_start(out=sb_bx, in_=bx.ap())
            sb_cur = consts.tile([W, C, H], bf16, tag="cur")
            nc.sync.dma_start(out=sb_cur, in_=curb.ap())

            # ---- PE warm-up during the DMA window -----------------------
            dummy = consts.tile([128, 448], bf16, tag="dummy")
            nc.vector.memset(dummy, 0.0)
            for i in range(_N_WARMUP):
                pool, nm = (psA, "psa_t") if i % 2 == 0 else (psB, "psb_t")
                ps = pool.tile([W, 2, 512], fp32, name=nm)
                nc.tensor.matmul(ps[:, 0, 0:448], dummy[:, 0:W], dummy,
                                 start=True, stop=True)

            # ---- stage 1: y-interpolation -------------------------------
            tps = []
            for v in range(V):
                tps.append(consts.tile([W, C, DLOC * H], bf16,
                                       name=f"tp{v}", tag=f"tp{v}"))
            tpf = [t.rearrange("p a b -> p (a b)") for t in tps]
            idx = 0
            for v in range(V):
                rhs = sb_ay[:, v * DLOC * H:(v + 1) * DLOC * H]
                for p in range(C // 2):
                    use_act = _S1_ACT[idx]
                    pool, nm = (psA, "psa_t") if use_act else (psB, "psb_t")
                    ps = pool.tile([W, 2, 512], fp32, name=nm)
                    for cc in range(2):
                        c = 2 * p + cc
                        lhsT = sb_src[:, (v * C + c) * W:(v * C + c + 1) * W]
                        nc.tensor.matmul(ps[:, cc, :], lhsT, rhs,
                                         start=True, stop=True)
                    dst = tpf[v][:, 2 * p * 512:(2 * p + 2) * 512]
                    psf = ps.rearrange("p a b -> p (a b)")
                    if use_act:
                        nc.scalar.copy(dst, psf)
                    else:
                        nc.vector.tensor_copy(dst, psf)
                    idx += 1

            # ---- stage 2: x-interpolation + view sum + channel dot ------
            obig = consts.tile([W, DLOC, H], fp32, tag="obig")
            for d in range(DLOC):
                pool, nm = (psA, "psa_t") if d % 2 == 0 else (psB, "psb_t")
                ps2r = pool.tile([W, 2, 512], fp32, name=nm)
                ps2 = ps2r.rearrange("p a (c b) -> p (a c) b", b=H)
                for v in range(V):
                    lhsT = sb_bx[:, (v * DLOC + d) * W:(v * DLOC + d + 1) * W]
                    for h in range(2):
                        rhs = tps[v][:, h * 8:(h + 1) * 8, d * H:(d + 1) * H]
                        nc.tensor.matmul(ps2[:, h * 8:(h + 1) * 8, :],
                                         lhsT, rhs,
                                         start=(v == 0), stop=(v == V - 1))
                o = obig[:, d, :]
                if d >= DLOC - 2:
                    # late planes: single-engine chain, no cross-engine hops
                    q1 = q1p.tile([W, C, H], bf16)
                    nc.vector.tensor_mul(q1, ps2, sb_cur)
                    ga = rp.tile([W, C // 2, H], bf16)
                    nc.vector.tensor_add(ga, q1[:, 0:8, :], q1[:, 8:16, :])
                    gb = rp2.tile([W, C // 4, H], bf16)
                    nc.vector.tensor_add(gb, ga[:, 0:4, :], ga[:, 4:8, :])
                    gc = rp.tile([W, 2, H], fp32, name="gc")
                    nc.vector.tensor_add(gc, gb[:, 0:2, :], gb[:, 2:4, :])
                    nc.vector.tensor_add(o, gc[:, 0, :], gc[:, 1, :])
                else:
                    q0 = q0p.tile([W, C, H], bf16)
                    nc.scalar.copy(q0, ps2)
                    q1 = q1p.tile([W, C, H], bf16)
                    nc.vector.tensor_mul(q1, q0, sb_cur)
                    ga = rp.tile([W, C // 2, H], bf16)
                    nc.vector.tensor_add(ga, q1[:, 0:8, :], q1[:, 8:16, :])
                    gb = rp2.tile([W, C // 4, H], bf16)
                    nc.gpsimd.tensor_add(gb, ga[:, 0:4, :], ga[:, 4:8, :])
                    gc = rp.tile([W, 2, H], fp32, name="gc")
                    nc.gpsimd.tensor_add(gc, gb[:, 0:2, :], gb[:, 2:4, :])
                    nc.gpsimd.tensor_add(o, gc[:, 0, :], gc[:, 1, :])


            nc.sync.dma_start(out=out.ap()[0:48], in_=obig[0:48])
            nc.scalar.dma_start(out=out.ap()[48:96], in_=obig[48:96])

    nc.compile()
    return nc


def _get_nc():
    if "nc" not in _CACHE:
        _CACHE["nc"] = _build_nc()
    return _CACHE["nc"]


# --------------------------------------------------------------------------
# Host-side geometry
# --------------------------------------------------------------------------
def _depth_planes(min_depth, max_depth):
    """Mimic the reference's fp32 arithmetic."""
    ramp = np.linspace(0.0, 1.0, D, dtype=np.float32)
    inv_min = (np.float32(1.0) / np.float32(min_depth)).astype(np.float32)
    inv_max = (np.float32(1.0) / np.float32(max_depth)).astype(np.float32)
    return (np.float32(1.0) /
            (inv_min + (inv_max - inv_min) * ramp).astype(np.float32))


def _is_separable(src_extrinsics, src_Ks, cur_invK):
    E = src_extrinsics[0]          # (V,4,4)
    K = src_Ks[0]                  # (V,4,4)
    iK = cur_invK[0]               # (4,4)
    eye3 = np.eye(3, dtype=E.dtype)
    for v in range(V):
        if not np.array_equal(E[v, :3, :3], eye3):
            return False
        if not np.array_equal(E[v, 3], np.array([0, 0, 0, 1], dtype=E.dtype)):
            return False
        k = K[v]
        if not (k[0, 1] == 0 and k[0, 3] == 0 and k[1, 0] == 0 and k[1, 3] == 0
                and np.array_equal(k[2], np.array([0, 0, 1, 0], dtype=K.dtype))):
            return False
    if not (iK[0, 1] == 0 and iK[1, 0] == 0 and iK[2, 0] == 0
            and iK[2, 1] == 0 and iK[2, 2] == 1):
        return False
    return True


def _coords(src_extrinsics, src_Ks, cur_invK, depths):
    """Per-(view, plane) 1-D sample coordinates: x[v,d,px], y[v,d,py]."""
    E = src_extrinsics[0].astype(np.float64)
    K = src_Ks[0].astype(np.float64)
    iK = cur_invK[0].astype(np.float64)
    i00, i02 = iK[0, 0], iK[0, 2]
    i11, i12 = iK[1, 1], iK[1, 2]
    px = np.arange(W, dtype=np.float64) + 0.5
    py = np.arange(H, dtype=np.float64) + 0.5
    xcs = np.empty((V, D, W), np.float64)
    ycs = np.empty((V, D, H), np.float64)
    for v in range(V):
        k00, k02 = K[v, 0, 0], K[v, 0, 2]
        k11, k12 = K[v, 1, 1], K[v, 1, 2]
        tx, ty, tz = E[v, 0, 3], E[v, 1, 3], E[v, 2, 3]
        for d in range(D):
            Dd = float(depths[d])
            z32 = np.float32(depths[d]) + np.float32(tz)        # ref fp32 z
            if not (z32 > 0):
                xcs[v, d] = OOB
                ycs[v, d] = OOB
                continue
            Zs = float(np.float32(z32 + np.float32(EPS)))
            rx = i00 * px + i02
            ry = i11 * py + i12
            u = (k00 * rx * Dd + k02 * Dd + k00 * tx + k02 * tz) / Zs
            vv = (k11 * ry * Dd + k12 * Dd + k11 * ty + k12 * tz) / Zs
            xcs[v, d] = np.clip(np.nan_to_num(u - 0.5, nan=OOB,
                                              posinf=OOB, neginf=-OOB),
                                -OOB, OOB)
            ycs[v, d] = np.clip(np.nan_to_num(vv - 0.5, nan=OOB,
                                              posinf=OOB, neginf=-OOB),
                                -OOB, OOB)
    return xcs.astype(np.float32), ycs.astype(np.float32)


def _hats(coords_flat, n_taps):
    """hat(t - k) = relu(1 - |t - k|) as [n_taps, len(coords)] bf16."""
    from ml_dtypes import bfloat16
    taps = np.arange(n_taps, dtype=np.float32)[:, None]
    h = np.maximum(np.float32(0.0),
                   np.float32(1.0) - np.abs(coords_flat[None, :] - taps))
    return np.ascontiguousarray(h.astype(bfloat16))


# --------------------------------------------------------------------------
# Exact numpy fallback (general geometry)
# --------------------------------------------------------------------------
def _reference_numpy(cur_feats, src_feats, src_extrinsics, src_Ks, cur_invK,
                     min_depth, max_depth):
    f32 = np.float32
    N = H * W
    dp = _depth_planes(min_depth.reshape(-1)[0], max_depth.reshape(-1)[0])
    xx, yy = np.meshgrid(np.arange(W, dtype=f32) + 0.5,
                         np.arange(H, dtype=f32) + 0.5)
    pix = np.stack([xx.ravel(), yy.ravel(), np.ones(N, f32)], 0)       # (3,N)
    rays = cur_invK[0, :3, :3].astype(f32) @ pix                       # (3,N)
    world = rays[None] * dp[:, None, None]                             # (D,3,N)
    world4 = np.concatenate([world, np.ones((D, 1, N), f32)], 1)       # (D,4,N)
    P = np.einsum("vij,vjk->vik", src_Ks[0], src_extrinsics[0])[:, :3]  # (V,3,4)
    cam = np.einsum("vij,djn->vdin", P, world4).astype(f32)            # (V,D,3,N)
    z = cam[:, :, 2]
    u = cam[:, :, 0] / (z + f32(EPS))
    vv = cam[:, :, 1] / (z + f32(EPS))
    x = (u - 0.5).astype(f32).reshape(V, D * N)
    y = (vv - 0.5).astype(f32).reshape(V, D * N)
    out = np.zeros((D, H, W), f32)
    cur = cur_feats[0].reshape(C, N)                                   # (C,N)
    for v in range(V):
        f = src_feats[0, v].reshape(C, N)
        x0 = np.floor(x[v])
        y0 = np.floor(y[v])
        acc = np.zeros((C, D * N), f32)
        for dx in (0.0, 1.0):
            for dy in (0.0, 1.0):
                xi = x0 + dx
                yi = y0 + dy
                wgt = (1.0 - np.abs(x[v] - xi)) * (1.0 - np.abs(y[v] - yi))
                valid = ((xi >= 0) & (xi < W) & (yi >= 0) & (yi < H))
                idx = (np.clip(yi, 0, H - 1) * W +
                       np.clip(xi, 0, W - 1)).astype(np.int64)
                acc += f[:, idx] * (wgt * valid.astype(f32))[None]
        dot = (acc.reshape(C, D, N) *
               cur[:, None, :]).sum(0)                                 # (D,N)
        mask = (z[v] > 0).astype(f32)                                  # (D,N)
        out += (dot * mask).reshape(D, H, W)
    return out[None].astype(np.float32)


# --------------------------------------------------------------------------
# Entry points
# --------------------------------------------------------------------------
def _prepare_inputs(cur_feats, src_feats, src_extrinsics, src_Ks, cur_invK,
                    min_depth, max_depth):
    from ml_dtypes import bfloat16
    dp = _depth_planes(min_depth.reshape(-1)[0], max_depth.reshape(-1)[0])
    xcs, ycs = _coords(src_extrinsics, src_Ks, cur_invK, dp)
    # src: (V,C,H,W) -> [H, V*C*W] bf16
    srcb = np.ascontiguousarray(
        src_feats[0].transpose(2, 0, 1, 3).reshape(H, V * C * W)
    ).astype(bfloat16)
    # cur: (C,H,W) -> [W, C, H]
    curb = np.ascontiguousarray(
        cur_feats[0].transpose(2, 0, 1)).astype(bfloat16)
    in_maps = []
    for k in range(N_CORES):
        sl = slice(k * DLOC, (k + 1) * DLOC)
        ayk = _hats(ycs[:, sl, :].reshape(-1), H)      # [H, V*DLOC*H]
        bxk = _hats(xcs[:, sl, :].reshape(-1), W)      # [W, V*DLOC*W]
        in_maps.append({
            "srcb": srcb,
            "ay": ayk,
            "bx": bxk,
            "curb": curb,
        })
    return in_maps


def _run(inputs, trace=False):
    from concourse.bass_utils import run_bass_kernel_spmd
    nc = _get_nc()
    in_maps = _prepare_inputs(**inputs)
    res = run_bass_kernel_spmd(nc, in_maps, core_ids=list(range(N_CORES)),
                               trace=trace)
    parts = [res.results[k]["out"].transpose(1, 2, 0) for k in range(N_CORES)]
    out = np.concatenate(parts, 0)[None].astype(np.float32)
    return out, res


def kernel(cur_feats, src_feats, src_extrinsics, src_Ks, cur_invK,
           min_depth, max_depth):
    args = dict(cur_feats=np.asarray(cur_feats), src_feats=np.asarray(src_feats),
                src_extrinsics=np.asarray(src_extrinsics),
                src_Ks=np.asarray(src_Ks), cur_invK=np.asarray(cur_invK),
                min_depth=np.asarray(min_depth), max_depth=np.asarray(max_depth))
    if not _is_separable(args["src_extrinsics"], args["src_Ks"],
                         args["cur_invK"]):
        return _reference_numpy(**args)
    out, _ = _run(args)
    return out
